# revision 2
# baseline (speedup 1.0000x reference)
"""Bass/Trainium2 kernel for nn_ConflictDetector (pairwise conflict scorer).

Reference computation:
    e  = concat(subj_emb, rel_emb, obj_emb) @ proj_w.T + proj_b        [N, 64]
    hi = e @ w1a.T ; hj = e @ w1b.T                                    [N, 64]
    h   = relu(hi[:,None,:] + hj[None,:,:] + b1)                       [N, N, 64]
    h2  = relu(h @ w2.T + b2)                                          [N, N, 32]
    s   = sigmoid(h2 @ w3[0] + b3[0])                                  [N, N]
    out = triu(s, k=1)

Strategy (data-parallel over pair rows, 8 cores):
  * Dedup claims on host (U ~1332 distinct of 2048); score the U x U grid
    of distinct claims on-device, gather back to [N, N] + triu on host.
  * Embedding + first linear run on host (tiny); device does the O(U^2)
    pairwise MLP in fp8e4m3 with DoubleRow matmuls (2 rows/cycle).
  * Grid tiled into 64-row x 448-col units distributed round-robin over
    8 cores; per unit:
      relu1: 32 ops of [128,448] (2 i x 64 d partitions), bf16 in ->
             fp8 out, engine per R1_ENG knob (DVE / ACT / GpSimd).
      mm1  : 8 DoubleRow matmuls (one per 4-i quad): lhsT = 2-slot
             blockdiag w2^T [128,2,128] fp8, rhs [128,2,448] fp8 ->
             h2 [128,448] f32 PSUM (2 quads per bank-aligned [128,1024]).
      relu2: 8 ops of [128,896] PSUM -> fp8 SBUF (+b2), engine per R2_ENG.
      mm2  : 8 DoubleRow matmuls accumulating [64,448] raw scores.
      evac : tensor_copy PSUM -> bf16 SBUF, DMA out; sigmoid+b3 on host.
"""

import numpy as np
import ml_dtypes

N = 2048
D = 64
IB = 64      # i-block rows per unit
JW = 448     # j-width per unit
N_CORES = 8
BF16 = ml_dtypes.bfloat16
FP8 = ml_dtypes.float8_e4m3

# Engine assignment knobs:
# R1_ENG[p]: engine for relu1 of pair p (0..31) in every unit:
#   'v' = DVE tensor_scalar, 'a' = ACT activation, 'p' = GpSimd tensor_scalar
# R2_ENG[gg]: engine for relu2 of group-pair gg (0..7) in every unit.
# EVAC_ENG: engine for the score PSUM->SBUF copy.
R1_ENG = "v" * 32
R2_ENG = "a" * 8
EVAC_ENG = "v"

_CACHE = {}


def _build_bass(U):
    """U = units per core."""
    import concourse.bacc as bacc
    import concourse.mybir as mybir
    from concourse.tile import TileContext

    bf16 = mybir.dt.bfloat16
    fp8 = mybir.dt.float8e4
    f32 = mybir.dt.float32

    nc = bacc.Bacc(target_bir_lowering=False)

    hj_pack = nc.dram_tensor("hj_pack", [128, U * JW], bf16, kind="ExternalInput")
    # cw: packed fp8 weights [w2dr (256) | w3dr (8*128)]
    cw = nc.dram_tensor("cw", [128, 256 + 8 * 128], fp8, kind="ExternalInput")
    # cf0: b2p | unit-0 bias cols (tiny, leads the ring); cfr: the rest.
    cf0 = nc.dram_tensor("cf0", [128, 1 + 32], f32, kind="ExternalInput")
    cfr = nc.dram_tensor("cfr", [128, max(U - 1, 1) * 32], f32, kind="ExternalInput")
    out = nc.dram_tensor("out", [U * IB, JW], bf16, kind="ExternalOutput")

    add = mybir.AluOpType.add
    vmax = mybir.AluOpType.max
    Relu = mybir.ActivationFunctionType.Relu
    DR = mybir.MatmulPerfMode.DoubleRow

    with TileContext(nc) as tc:
        with (
            tc.tile_pool(name="const", bufs=1) as cpool,
            tc.tile_pool(name="rhs1", bufs=8) as rhs1pool,
            tc.tile_pool(name="rhs2", bufs=4) as rhs2pool,
            tc.tile_pool(name="sout", bufs=4) as soutpool,
            tc.tile_pool(name="ps1", bufs=3, space="PSUM") as ps1pool,
            tc.tile_pool(name="ps2", bufs=2, space="PSUM") as ps2pool,
        ):
            # relu1-critical inputs (tiny cf0, hj unit 0) lead the Sync ring;
            # the weights ride the Scalar ring in parallel; bulk bias columns
            # and remaining hj units follow.
            cf0_sb = cpool.tile([128, 1 + 32], f32)
            nc.sync.dma_start(out=cf0_sb[:], in_=cf0[:])
            hj_sb = cpool.tile([128, U * JW], bf16)
            nc.sync.dma_start(out=hj_sb[:, 0:JW], in_=hj_pack[:, 0:JW])
            cw_sb = cpool.tile([128, 256 + 8 * 128], fp8)
            nc.scalar.dma_start(out=cw_sb[:], in_=cw[:])
            w2dr_sb = cw_sb[:, 0:256]
            w3dr_sb = cw_sb[:, 256:]
            b2p_sb = cf0_sb[:, 0:1]
            if U > 1:
                nc.sync.dma_start(
                    out=hj_sb[:, JW : 2 * JW], in_=hj_pack[:, JW : 2 * JW]
                )
            cfr_sb = cpool.tile([128, max(U - 1, 1) * 32], f32)
            nc.sync.dma_start(out=cfr_sb[:], in_=cfr[:])
            for u in range(2, U, 2):
                hi_u = min(u + 2, U)
                nc.sync.dma_start(
                    out=hj_sb[:, u * JW : hi_u * JW],
                    in_=hj_pack[:, u * JW : hi_u * JW],
                )

            def cp_col(u, p):
                if u == 0:
                    return cf0_sb[:, 1 + p : 2 + p]
                return cfr_sb[:, (u - 1) * 32 + p : (u - 1) * 32 + p + 1]

            def eng(ch):
                return {"v": nc.vector, "a": nc.scalar, "p": nc.gpsimd}[ch]

            for u in range(U):
                hj_u = hj_sb[:, u * JW : (u + 1) * JW]
                s_ps = ps2pool.tile([64, JW], f32)
                for gg in range(8):
                    # Two quads (4 i's each) at bank-aligned 512-col slots.
                    h2_ps = ps1pool.tile([128, 1024], f32)
                    rhs1s = []
                    for g2 in range(2):
                        g = 2 * gg + g2  # quad index (0..15)
                        rhs1 = rhs1pool.tile([128, 2 * JW], fp8)
                        rhs1s.append(rhs1)
                        for h in range(2):
                            p = 2 * g + h  # pair index within unit (0..31)
                            e = R1_ENG[p]
                            dst = rhs1[:, h * JW : (h + 1) * JW]
                            if e == "a":
                                nc.scalar.activation(
                                    dst, hj_u, Relu, bias=cp_col(u, p), scale=1.0
                                )
                            else:
                                eng(e).tensor_scalar(
                                    dst, hj_u, cp_col(u, p), 0.0, add, vmax
                                )
                        nc.tensor.matmul(
                            h2_ps[:, g2 * 512 : g2 * 512 + JW],
                            lhsT=w2dr_sb.rearrange("p (two f) -> p two f", two=2),
                            rhs=rhs1[:].rearrange("p (two j) -> p two j", two=2),
                            start=True,
                            stop=True,
                            perf_mode=DR,
                        )
                    rhs2 = rhs2pool.tile([128, 2 * JW], fp8)
                    h2_rd = h2_ps[:].rearrange("p (g j) -> p g j", g=2)[:, :, 0:JW]
                    rhs2_wr = rhs2[:].rearrange("p (g j) -> p g j", g=2)
                    e2 = R2_ENG[gg]
                    if e2 == "a":
                        nc.scalar.activation(
                            rhs2_wr, h2_rd, Relu, bias=b2p_sb[:, 0:1], scale=1.0
                        )
                    else:
                        eng(e2).tensor_scalar(
                            rhs2_wr, h2_rd, b2p_sb[:, 0:1], 0.0, add, vmax
                        )
                    nc.tensor.matmul(
                        s_ps[:],
                        lhsT=w3dr_sb[:, gg * 128 : (gg + 1) * 128].rearrange(
                            "p (two f) -> p two f", two=2
                        ),
                        rhs=rhs2[:].rearrange("p (two j) -> p two j", two=2),
                        start=(gg == 0),
                        stop=(gg == 7),
                        perf_mode=DR,
                    )
                s_sb = soutpool.tile([64, JW], bf16)
                # Raw scores out; host applies sigmoid+b3.
                if EVAC_ENG == "a":
                    nc.scalar.activation(
                        s_sb[:], s_ps[:], mybir.ActivationFunctionType.Copy
                    )
                else:
                    eng(EVAC_ENG).tensor_copy(out=s_sb[:], in_=s_ps[:])
                nc.sync.dma_start(out=out[u * IB : (u + 1) * IB, :], in_=s_sb[:])

    nc.finalize()
    return nc


def _get_nc(U):
    key = ("nc", U)
    if key not in _CACHE:
        _CACHE[key] = _build_bass(U)
    return _CACHE[key]


def kernel(
    subj_idx, rel_idx, obj_idx, subj_table, rel_table, obj_table,
    proj_w, proj_b, w1, b1, w2, b2, w3, b3,
):
    from concourse.bass_utils import run_bass_kernel_spmd

    subj_idx = np.asarray(subj_idx)
    rel_idx = np.asarray(rel_idx)
    obj_idx = np.asarray(obj_idx)
    subj_table = np.asarray(subj_table, np.float32)
    rel_table = np.asarray(rel_table, np.float32)
    obj_table = np.asarray(obj_table, np.float32)
    proj_w = np.asarray(proj_w, np.float32)
    proj_b = np.asarray(proj_b, np.float32)
    w1 = np.asarray(w1, np.float32)
    b1 = np.asarray(b1, np.float32)
    w2 = np.asarray(w2, np.float32)
    b2 = np.asarray(b2, np.float32)
    w3 = np.asarray(w3, np.float32)
    b3 = np.asarray(b3, np.float32)

    # ---- host: dedup claims ----
    key = (subj_idx.astype(np.int64) * rel_table.shape[0] + rel_idx) * obj_table.shape[
        0
    ] + obj_idx
    ukey, inv = np.unique(key, return_inverse=True)
    Uq = len(ukey)
    us = (ukey // (rel_table.shape[0] * obj_table.shape[0])).astype(np.int64)
    ur = ((ukey // obj_table.shape[0]) % rel_table.shape[0]).astype(np.int64)
    uo = (ukey % obj_table.shape[0]).astype(np.int64)

    n_ib = (Uq + IB - 1) // IB
    n_ju = (Uq + JW - 1) // JW
    units = [(b, j) for b in range(n_ib) for j in range(n_ju)]
    units_per_core = (len(units) + N_CORES - 1) // N_CORES
    n_slots = N_CORES * units_per_core
    units = units + [units[0]] * (n_slots - len(units))  # pad with dummies
    ipad = n_ib * IB
    jpad = n_ju * JW

    # ---- host: embedding + first linear for unique claims (tiny) ----
    combined = np.concatenate(
        [subj_table[us], rel_table[ur], obj_table[uo]], axis=-1
    )  # [Uq, 192]
    e = combined @ proj_w.T + proj_b  # [Uq, 64]
    w1a, w1b = w1[:, :D], w1[:, D:]
    hi = e @ w1a.T
    hj = e @ w1b.T
    C = np.zeros((ipad, D), np.float32)
    C[:Uq] = hi + b1  # per-row bias for relu1
    hjT = np.zeros((D, jpad), np.float32)
    hjT[:, :Uq] = hj.T

    # ---- static packed weights (same for all cores) ----
    # w2dr [128, 2 slots, 128 outs]: slot 0 -> quad members 0,1 (out 0:64),
    # slot 1 -> members 2,3 (out 64:128); within a slot, partitions 0:64 are
    # the first member's 64 d dims, 64:128 the second's.
    w2dr = np.zeros((128, 2, 128), np.float32)
    w2dr[:64, 0, 0:32] = w2.T  # [d, k2]
    w2dr[64:, 0, 32:64] = w2.T
    w2dr[:64, 1, 64:96] = w2.T
    w2dr[64:, 1, 96:128] = w2.T
    w2dr = w2dr.reshape(128, 256)

    # w3dr [128, 8 gg, 2 slots, 64 outs]: slot s of gg handles quad
    # g = 2*gg+s; member q (partitions 32q:32q+32 = its k dims) scores land
    # on out row 4g+q.
    w3dr = np.zeros((128, 8, 2, 64), np.float32)
    for gg in range(8):
        for s in range(2):
            g = 2 * gg + s
            for q in range(4):
                w3dr[32 * q : 32 * (q + 1), gg, s, 4 * g + q] = w3[0]
    w3dr = w3dr.reshape(128, 8 * 128)

    cw = np.concatenate([w2dr, w3dr], axis=1).astype(FP8)  # [128, 256+1024]
    b2p = np.tile(b2, 4).reshape(128, 1).astype(np.float32)

    # ---- per-core packs ----
    in_maps = []
    for c in range(N_CORES):
        units_c = units[c::N_CORES]
        hj_pack = np.zeros((128, units_per_core * JW), np.float32)
        cp_pack = np.zeros((128, units_per_core * 32), np.float32)
        for u, (b, ju) in enumerate(units_c):
            blk = hjT[:, ju * JW : (ju + 1) * JW]
            hj_pack[:64, u * JW : (u + 1) * JW] = blk
            hj_pack[64:, u * JW : (u + 1) * JW] = blk
            for p in range(32):
                cp_pack[:64, u * 32 + p] = C[IB * b + 2 * p]
                cp_pack[64:, u * 32 + p] = C[IB * b + 2 * p + 1]
        cf0 = np.concatenate([b2p, cp_pack[:, :32]], axis=1)  # [128, 33]
        cfr = np.ascontiguousarray(cp_pack[:, 32:])
        if cfr.shape[1] == 0:
            cfr = np.zeros((128, 32), np.float32)
        in_maps.append(
            {
                "hj_pack": hj_pack.astype(BF16),
                "cw": cw,
                "cf0": cf0,
                "cfr": cfr,
            }
        )

    nc = _get_nc(units_per_core)
    res = run_bass_kernel_spmd(
        nc, in_maps, core_ids=list(range(N_CORES)), **_CACHE.get("run_kwargs", {})
    )
    _CACHE["last_result"] = res

    # ---- gather: unit tiles -> unique grid -> full [N, N] -> triu ----
    ugrid = np.zeros((ipad, jpad), np.float32)
    seen = set()
    for c in range(N_CORES):
        units_c = units[c::N_CORES]
        out_c = res.results[c]["out"].reshape(units_per_core, IB, JW)
        for u, (b, ju) in enumerate(units_c):
            if (b, ju) in seen:
                continue  # dummy duplicate
            seen.add((b, ju))
            blk = out_c[u].astype(np.float64)
            # Scores leave the device pre-sigmoid; apply sigmoid+b3 here.
            blk = 1.0 / (1.0 + np.exp(-(blk + b3[0])))
            ugrid[b * IB : (b + 1) * IB, ju * JW : (ju + 1) * JW] = blk.astype(
                np.float32
            )
    scores = ugrid[np.ix_(inv, inv)]
    return np.triu(scores, k=1)
